# revision 15
# baseline (speedup 1.0000x reference)
"""DLSTM Trainium2 kernel.

Problem: T=512 steps of an LSTM-like recurrence over batch B=128,
I=H=512.  Sharding: data-parallel over batch across 8 NeuronCores
(B_local=16 per core), weights replicated; no cross-device
communication inside the recurrence.

Per-core structure:
  Phase A (batched pre-pass): u[t] = x_t @ W0x.T + b0  for all t
     (the x-contribution to the first layer is recurrence-independent)
  Phase B (sequential loop over t):
     h1   = relu(u_t + c @ W0c.T)
     gates= h1 @ W1g.T + b1g  (a,b,f,i interleaved by H-chunk)
     c    = sigmoid(i) * (relu(a) - relu(b)) + sigmoid(f) * c
     hx   = relu(h1 @ W1h.T + b1h)  -- batched per UNROLL-block from an
            SBUF ring of the steps' transposed h1, off the critical chain
Matmuls keep the small per-core batch stationary and stream the
(replicated) weights as the MOVING operand in float32r (1 cycle/row vs 4
for fp32).  u_t and the gate biases are folded into the PSUM
accumulation as extra matmuls (identity / ones stationary) so the
scalar engine consumes PSUM directly and the vector engine stays off
the critical chain.  Layout transposes run on the PE ([16,128] tiles).
"""

import numpy as np

T, B, I, H = 512, 128, 512, 512
N_CORES = 8
BL = B // N_CORES  # 16 local batch
ROWS = T * BL  # 8192 flattened (t, b) rows per core
UNROLL = 8  # recurrence steps per For_i body
PRE_UNROLL = 8  # pre-pass m-tiles per For_i body

_cache = {}


def _build(sim_pre_tiles=None, sim_steps=None):
    # sim_*: when set, emit fully-unrolled python loops (no For_i) with that
    # many pre-pass m-tile-groups / recurrence step-groups, for cost-model
    # simulation.
    from contextlib import contextmanager

    from concourse import bacc, mybir, tile
    from concourse.bass import ds

    f32 = mybir.dt.float32
    f32r = mybir.dt.float32r
    AF = mybir.ActivationFunctionType
    OP = mybir.AluOpType

    nc = bacc.Bacc("TRN2", target_bir_lowering=False, debug=False)

    @contextmanager
    def _iters(tc, n, unrolled_n):
        if unrolled_n is not None:
            yield range(unrolled_n)
        else:
            with tc.For_i(0, n, 1) as iv:
                yield [iv]

    # ---- DRAM I/O ----
    x_d = nc.dram_tensor("x", [ROWS, I], f32, kind="ExternalInput")
    c0_d = nc.dram_tensor("c0", [BL, H], f32, kind="ExternalInput")
    w0xt_d = nc.dram_tensor("w0xt", [I, H], f32r, kind="ExternalInput")
    w0ct_d = nc.dram_tensor("w0ct", [H, H], f32r, kind="ExternalInput")
    wg_d = nc.dram_tensor("wg", [H, 4 * H], f32r, kind="ExternalInput")
    wh_d = nc.dram_tensor("wh", [H, H], f32r, kind="ExternalInput")
    b0row_d = nc.dram_tensor("b0row", [1, H], f32r, kind="ExternalInput")
    bg_d = nc.dram_tensor("bg", [1, 4 * H], f32r, kind="ExternalInput")
    bh_d = nc.dram_tensor("bh", [1, H], f32r, kind="ExternalInput")
    id16_d = nc.dram_tensor("id16", [BL, BL], f32, kind="ExternalInput")
    id16r_d = nc.dram_tensor("id16r", [BL, BL], f32r, kind="ExternalInput")
    ones1_d = nc.dram_tensor("ones1", [1, 128], f32r, kind="ExternalInput")
    id128_d = nc.dram_tensor("id128", [128, 128], f32, kind="ExternalInput")

    hx_d = nc.dram_tensor("hx", [ROWS, H], f32, kind="ExternalOutput")
    cout_d = nc.dram_tensor("cout", [BL, H], f32, kind="ExternalOutput")

    KC = H // 128  # 4 contraction chunks

    with tile.TileContext(nc) as tc:
        with (
            tc.tile_pool(name="persist", bufs=1) as pp,
            tc.tile_pool(name="dram", bufs=1, space="DRAM") as dp,
        ):
            # persistent SBUF: weights, biases, state
            w0xt = pp.tile([128, KC * H], f32r)  # chunk k at cols [H*k, H*(k+1))
            w0ct = pp.tile([128, KC * H], f32r)
            wg = pp.tile([128, KC * 4 * H], f32r)  # chunk k at cols [4H*k ...)
            wh = pp.tile([128, KC * H], f32r)
            b0row = pp.tile([1, H], f32r)
            bg = pp.tile([1, 4 * H], f32r)
            bh = pp.tile([1, H], f32r)
            id16 = pp.tile([BL, BL], f32)
            id16r = pp.tile([BL, BL], f32r)
            ones1 = pp.tile([1, 128], f32r)
            id128 = pp.tile([128, 128], f32)
            c_m = [pp.tile([BL, 256], f32, name=f"c_m{_k}") for _k in range(2)]
            cT_j = [pp.tile([128, BL], f32r, name=f"cT_j{_k}") for _k in range(KC)]

            u_dram = dp.tile([ROWS, H], f32r)

            for k in range(KC):
                nc.sync.dma_start(w0xt[:, H * k : H * (k + 1)], w0xt_d[128 * k : 128 * (k + 1), :])
                nc.sync.dma_start(w0ct[:, H * k : H * (k + 1)], w0ct_d[128 * k : 128 * (k + 1), :])
                nc.sync.dma_start(wg[:, 4 * H * k : 4 * H * (k + 1)], wg_d[128 * k : 128 * (k + 1), :])
                nc.sync.dma_start(wh[:, H * k : H * (k + 1)], wh_d[128 * k : 128 * (k + 1), :])
            nc.sync.dma_start(b0row[:], b0row_d[:])
            nc.sync.dma_start(bg[:], bg_d[:])
            nc.sync.dma_start(bh[:], bh_d[:])
            nc.sync.dma_start(id16[:], id16_d[:])
            nc.sync.dma_start(id16r[:], id16r_d[:])
            nc.sync.dma_start(ones1[:], ones1_d[:])
            nc.sync.dma_start(id128[:], id128_d[:])

            # ---- Phase A: u = x @ W0x.T + b0  (batched over all rows) ----
            M_TILES = ROWS // 128  # 64
            with (
                tc.tile_pool(name="pre_sb", bufs=3) as psb,
                tc.tile_pool(name="pre_ps", bufs=2, space="PSUM") as pps,
            ):
                with _iters(tc, M_TILES // PRE_UNROLL, sim_pre_tiles) as _mis:
                  for mi in _mis:
                    for mu in range(PRE_UNROLL):
                        xa = psb.tile([128, I], f32, tag="xa")
                        nc.sync.dma_start(
                            xa[:], x_d[ds(mi * (128 * PRE_UNROLL) + 128 * mu, 128), :]
                        )
                        xT = psb.tile([128, KC * 128], f32r, tag="xT")
                        for k in range(KC):
                            ptr = pps.tile([128, 128], f32, tag="ptr")
                            nc.tensor.transpose(ptr[:], xa[:, 128 * k : 128 * (k + 1)], id128[:])
                            nc.vector.tensor_copy(xT[:, 128 * k : 128 * (k + 1)], ptr[:])
                        pu = pps.tile([128, H], f32, tag="pu")
                        nc.tensor.matmul(pu[:], ones1[:], b0row[:], start=True, stop=False)
                        for k in range(KC):
                            nc.tensor.matmul(
                                pu[:],
                                xT[:, 128 * k : 128 * (k + 1)],
                                w0xt[:, H * k : H * (k + 1)],
                                start=False,
                                stop=(k == KC - 1),
                            )
                        ua = psb.tile([128, H], f32r, tag="ua")
                        nc.vector.tensor_copy(ua[:], pu[:])
                        nc.sync.dma_start(
                            u_dram[ds(mi * (128 * PRE_UNROLL) + 128 * mu, 128), :], ua[:]
                        )

            # init c_j / cT_j from c0
            with (
                tc.tile_pool(name="ct_ps", bufs=2, space="PSUM") as cps,
                tc.tile_pool(name="ct_sb", bufs=1) as csb,
            ):
                c0_sb = csb.tile([BL, H], f32)
                nc.sync.dma_start(c0_sb[:], c0_d[:])
                for m in range(2):
                    nc.vector.tensor_copy(c_m[m][:], c0_sb[:, 256 * m : 256 * (m + 1)])
                for k in range(KC):
                    ptr = cps.tile([128, BL], f32, tag="ctr")
                    nc.tensor.transpose(ptr[:], c0_sb[:, 128 * k : 128 * (k + 1)], id16[:])
                    nc.vector.tensor_copy(cT_j[k][:], ptr[:])

            # ---- Phase B: the recurrence ----
            with (
                tc.tile_pool(name="lp_sb", bufs=3) as lsb,
                tc.tile_pool(name="hx_sb", bufs=2) as hsb,
                tc.tile_pool(name="ps_h1", bufs=2, space="PSUM") as ps_h1,
                tc.tile_pool(name="ps_tr", bufs=2, space="PSUM") as ps_tr,
                tc.tile_pool(name="ps_g", bufs=3, space="PSUM") as ps_g,
                tc.tile_pool(name="ps_hx", bufs=1, space="PSUM") as ps_hx,
            ):
                with _iters(tc, T // UNROLL, sim_steps) as _tis:
                  for ti in _tis:
                    # per-body ring of transposed h1 (for the batched hx block)
                    h1R = [hsb.tile([128, BL * UNROLL], f32r, tag=f"h1R{_k}", name=f"h1R{_k}") for _k in range(KC)]
                    for tu in range(UNROLL):
                        u_t = lsb.tile([BL, H], f32r, tag="u_t")
                        nc.sync.dma_start(
                            u_t[:], u_dram[ds(ti * (BL * UNROLL) + BL * tu, BL), :]
                        )
                        # h1 psum: u_t (identity mm) + c @ W0c.T
                        ph1 = ps_h1.tile([BL, H], f32, tag="ph1")
                        nc.tensor.matmul(ph1[:], id16r[:], u_t[:], start=True, stop=False)
                        for k in range(KC):
                            nc.tensor.matmul(
                                ph1[:],
                                cT_j[k][:],
                                w0ct[:, H * k : H * (k + 1)],
                                start=False,
                                stop=(k == KC - 1),
                            )
                        # chunked relu of h1 so each chunk's transpose can
                        # start without waiting for the full width
                        h1 = lsb.tile([BL, H], f32, tag="h1")
                        nc.scalar.activation(h1[:], ph1[:], AF.Relu)
                        for k in range(KC):
                            ptr = ps_tr.tile([128, BL], f32, tag="tr")
                            nc.tensor.transpose(ptr[:], h1[:, 128 * k : 128 * (k + 1)], id16[:])
                            nc.vector.tensor_copy(
                                h1R[k][:, BL * tu : BL * (tu + 1)], ptr[:]
                            )
                        # gate banks, H-halves: AB_m = [a_2m|a_2m+1|b_2m|b_2m+1]
                        # (one relu), FI_m = [f.. |i..] (one sigmoid)
                        for m in range(2):
                            pab = ps_g.tile([BL, 512], f32, tag="pg")
                            nc.tensor.matmul(
                                pab[:], ones1[:, 0:BL], bg[:, 1024 * m : 1024 * m + 512],
                                start=True, stop=False,
                            )
                            for k in range(KC):
                                nc.tensor.matmul(
                                    pab[:],
                                    h1R[k][:, BL * tu : BL * (tu + 1)],
                                    wg[:, 4 * H * k + 1024 * m : 4 * H * k + 1024 * m + 512],
                                    start=False,
                                    stop=(k == KC - 1),
                                )
                            pfi = ps_g.tile([BL, 512], f32, tag="pg")
                            nc.tensor.matmul(
                                pfi[:], ones1[:, 0:BL], bg[:, 1024 * m + 512 : 1024 * (m + 1)],
                                start=True, stop=False,
                            )
                            for k in range(KC):
                                nc.tensor.matmul(
                                    pfi[:],
                                    h1R[k][:, BL * tu : BL * (tu + 1)],
                                    wg[:, 4 * H * k + 1024 * m + 512 : 4 * H * k + 1024 * (m + 1)],
                                    start=False,
                                    stop=(k == KC - 1),
                                )
                            ab = lsb.tile([BL, 512], f32, tag="ab")
                            nc.scalar.activation(ab[:], pab[:], AF.Relu)
                            fi = lsb.tile([BL, 512], f32, tag="fi")
                            nc.scalar.activation(fi[:], pfi[:], AF.Sigmoid)
                            # f * c on the (otherwise idle) gpsimd engine
                            t2 = lsb.tile([BL, 256], f32, tag="t2")
                            nc.gpsimd.tensor_tensor(t2[:], fi[:, 0:256], c_m[m][:], OP.mult)
                            ncx = lsb.tile([BL, 256], f32, tag="ncx")
                            nc.vector.tensor_tensor(ncx[:], ab[:, 0:256], ab[:, 256:512], OP.subtract)
                            t1 = lsb.tile([BL, 256], f32, tag="t1")
                            nc.vector.tensor_tensor(t1[:], fi[:, 256:512], ncx[:], OP.mult)
                            nc.vector.tensor_tensor(c_m[m][:], t1[:], t2[:], OP.add)
                            # transpose the two c chunks of this half
                            for q in range(2):
                                k = 2 * m + q
                                ptr2 = ps_tr.tile([128, BL], f32, tag="tr")
                                nc.tensor.transpose(
                                    ptr2[:], c_m[m][:, 128 * q : 128 * (q + 1)], id16[:]
                                )
                                nc.vector.tensor_copy(cT_j[k][:], ptr2[:])
                    # ---- batched hx for this body's UNROLL steps ----
                    phx = ps_hx.tile([128, H], f32, tag="phx")
                    nc.tensor.matmul(phx[:], ones1[:], bh[:], start=True, stop=False)
                    for k in range(KC):
                        nc.tensor.matmul(
                            phx[:], h1R[k][:], wh[:, H * k : H * (k + 1)],
                            start=False, stop=(k == KC - 1),
                        )
                    hx = hsb.tile([128, H], f32, tag="hx")
                    nc.scalar.activation(hx[:], phx[:], AF.Relu)
                    nc.sync.dma_start(hx_d[ds(ti * (BL * UNROLL), BL * UNROLL), :], hx[:])

            # final cell state out
            with tc.tile_pool(name="fin", bufs=1) as fin:
                cfin = fin.tile([BL, H], f32)
                for m in range(2):
                    nc.vector.tensor_copy(cfin[:, 256 * m : 256 * (m + 1)], c_m[m][:])
                nc.sync.dma_start(cout_d[:], cfin[:])

    nc.compile()
    return nc


def _host_prep(W0, b0, W1, b1):
    f = np.float32
    W0xT = np.ascontiguousarray(W0[:, :I].T, dtype=f)  # [I, H]
    W0cT = np.ascontiguousarray(W0[:, I:].T, dtype=f)  # [H, H]
    aT = np.ascontiguousarray(W1[0 * H : 1 * H].T, dtype=f)
    bT = np.ascontiguousarray(W1[1 * H : 2 * H].T, dtype=f)
    hT = np.ascontiguousarray(W1[2 * H : 3 * H].T, dtype=f)
    fT = np.ascontiguousarray(W1[3 * H : 4 * H].T, dtype=f)
    iT = np.ascontiguousarray(W1[4 * H : 5 * H].T, dtype=f)
    Wg = np.empty((H, 4 * H), dtype=f)
    bg = np.empty((1, 4 * H), dtype=f)
    b1a, b1b = b1[0 * H : 1 * H], b1[1 * H : 2 * H]
    b1f, b1i = b1[3 * H : 4 * H], b1[4 * H : 5 * H]
    for m in range(2):
        sl = slice(256 * m, 256 * (m + 1))
        base = 1024 * m
        Wg[:, base + 0 : base + 256] = aT[:, sl]
        Wg[:, base + 256 : base + 512] = bT[:, sl]
        Wg[:, base + 512 : base + 768] = fT[:, sl]
        Wg[:, base + 768 : base + 1024] = iT[:, sl]
        bg[0, base + 0 : base + 256] = b1a[sl]
        bg[0, base + 256 : base + 512] = b1b[sl]
        bg[0, base + 512 : base + 768] = b1f[sl]
        bg[0, base + 768 : base + 1024] = b1i[sl]
    return {
        "w0xt": W0xT,
        "w0ct": W0cT,
        "wg": Wg,
        "wh": hT,
        "b0row": b0.astype(f).reshape(1, H),
        "bg": bg,
        "bh": b1[2 * H : 3 * H].astype(f).reshape(1, H),
        "id16": np.eye(BL, dtype=f),
        "id16r": np.eye(BL, dtype=f),
        "ones1": np.ones((1, 128), dtype=f),
        "id128": np.eye(128, dtype=f),
    }


def kernel(x, cell_state, W0, b0, W1, b1, _trace=False):
    from concourse import bass_utils

    x = np.asarray(x, dtype=np.float32)
    cell_state = np.asarray(cell_state, dtype=np.float32)
    W0 = np.asarray(W0, dtype=np.float32)
    b0 = np.asarray(b0, dtype=np.float32)
    W1 = np.asarray(W1, dtype=np.float32)
    b1 = np.asarray(b1, dtype=np.float32)

    if "nc" not in _cache:
        _cache["nc"] = _build()
    nc = _cache["nc"]

    shared = _host_prep(W0, b0, W1, b1)
    in_maps = []
    for c in range(N_CORES):
        bsl = slice(BL * c, BL * (c + 1))
        m = dict(shared)
        m["x"] = np.ascontiguousarray(x[:, bsl, :]).reshape(ROWS, I)
        m["c0"] = np.ascontiguousarray(cell_state[0, bsl, :])
        in_maps.append(m)

    res = bass_utils.run_bass_kernel_spmd(
        nc, in_maps, core_ids=list(range(N_CORES)), trace=_trace
    )
    if _trace:
        _cache["last_result"] = res

    outputs = np.empty((T, B, H), dtype=np.float32)
    c_final = np.empty((1, B, H), dtype=np.float32)
    for c in range(N_CORES):
        bsl = slice(BL * c, BL * (c + 1))
        outputs[:, bsl, :] = res.results[c]["hx"].reshape(T, BL, H)
        c_final[0, bsl, :] = res.results[c]["cout"]
    return outputs, c_final
